# revision 26
# baseline (speedup 1.0000x reference)
"""Trainium2 Bass kernel for nn_EnsembleNet (10-head MLP ensemble).

Math (per head h):
  h1 = relu(x @ W1[h] + b1[h])      x: [B, 129], W1: [129, 16]
  h2 = relu(h1 @ W2[h] + b2[h])     W2: [16, 16]
  out[h] = h2 @ W3[h] + b3[h]       W3: [16, 16] -> [10, B, 16]

Strategy (data parallel over 8 cores, B=500000 -> 62500/core, padded to
63488 = 124 tiles x 512 samples):
  - Host splits x into xm=[B,128] (features 0..127) and xl=[1,B] (feature
    128) so the 129-feature contraction becomes one K=128 matmul plus one
    K=1 accumulate whose moving operand loads directly as a [1, 512] row.
  - Heads 0-7 ("A", 8*16=128 outputs) computed in the transposed domain:
    PE-transpose x tiles to [128 feat, 512 batch], then chained block-
    diagonal matmuls (float32r, 1 cyc/row) with fused bias+relu on ACT,
    PE-transpose the result back to batch-major and DMA out.
  - Heads 8-9 ("B", 32 outputs) are packed 4 sub-tiles deep on the
    partition axis (4 x 32 = 128) per 2048-sample super-tile so vector
    ops and matmuls run at full width; layer 1 places each sub-tile's
    strip via zero-padded M=128 weight variants accumulating into one
    packed psum bank (fp32r matmuls require dst base partition 0).
  - Device writes custom layouts outA [496,128,128] / outB [31,4,128,128]
    (>=512B contiguous per DMA descriptor); host permutes (64B-granular)
    into [10, B, 16].
  - Inputs load on the SP HWDGE ring, outputs store on the ACT HWDGE
    ring; one merged DMA per tile each way.
  Measured: ~553 us/core on 8 axon trn2 cores (repeat-loop difference
  method), scale-relative absmax err 2.8e-4 vs fp32 reference.
"""

import os
from contextlib import ExitStack

import numpy as np

import concourse.bass as bass
import concourse.mybir as mybir
import concourse.tile as tile
from concourse import bacc
from concourse.bass_utils import run_bass_kernel_spmd
from concourse.masks import make_identity

F32 = mybir.dt.float32
F32R = mybir.dt.float32r

N_CORES = 8
BATCH = 500000
SHARD = BATCH // N_CORES  # 62500
TILE = 512  # samples per tile (psum bank = 512 f32)
SUPER = 4 * TILE  # 2048, B-group packing unit
N_TILES = 124  # ceil(62500/512) -> pad to 124
PAD = N_TILES * TILE  # 63488
N_SUPERS = N_TILES // 4  # 31

NHEADS = 10
HID = 16
SKIP = 16
IN_DIM = 129


def _block_diag(mats):
    n = len(mats)
    r, c = mats[0].shape
    out = np.zeros((n * r, n * c), dtype=mats[0].dtype)
    for i, m in enumerate(mats):
        out[i * r : (i + 1) * r, i * c : (i + 1) * c] = m
    return out


def _pack_weights(W1, b1, W2, b2, W3, b3):
    """Host-side packing into the SBUF layouts the kernel expects."""
    W1 = np.asarray(W1, np.float32)
    W2 = np.asarray(W2, np.float32)
    W3 = np.asarray(W3, np.float32)
    b1 = np.asarray(b1, np.float32)
    b2 = np.asarray(b2, np.float32)
    b3 = np.asarray(b3, np.float32)

    d = {}
    # L1 A: lhsT [K=128 feat, M=128 (h,o)]
    d["w1a"] = np.ascontiguousarray(W1[:8, :128, :].transpose(1, 0, 2).reshape(128, 128))
    d["wla"] = np.ascontiguousarray(W1[:8, 128, :].reshape(1, 128))
    # L1 B: zero-padded M=128 variants, chunk c owns columns c*32..(c+1)*32
    # (fp32r matmul requires dst psum base partition 0, so each chunk's
    # [K,32] result is placed via its weight columns instead of col-tiling)
    w1b32 = W1[8:, :128, :].transpose(1, 0, 2).reshape(128, 32)
    wlb32 = W1[8:, 128, :].reshape(32)
    w1bs = np.zeros((4, 128, 128), np.float32)
    wlbs = np.zeros((4, 128), np.float32)
    for c in range(4):
        w1bs[c, :, c * 32 : (c + 1) * 32] = w1b32
        wlbs[c, c * 32 : (c + 1) * 32] = wlb32
    d["w1bs"] = w1bs
    d["wlbs"] = wlbs
    # bf16 hi/lo splits for the DMA-transpose input path
    import ml_dtypes

    def split16(m):
        hi = m.astype(ml_dtypes.bfloat16)
        lo = (m - hi.astype(np.float32)).astype(ml_dtypes.bfloat16)
        return hi, lo

    d["w1a_h"], d["w1a_l"] = split16(d["w1a"])
    d["w1bs_h"], d["w1bs_l"] = split16(w1bs)
    d["wla_h"] = d["wla"].astype(ml_dtypes.bfloat16)
    d["wlbs_h"] = wlbs.astype(ml_dtypes.bfloat16)
    # L2: block diag [in (h,i), out (h,o)]
    d["w2a"] = _block_diag([W2[h] for h in range(8)])
    w2b1 = _block_diag([W2[8], W2[9]])  # [32, 32]
    d["w2b"] = _block_diag([w2b1] * 4)  # [128, 128] over (c, g)
    d["w3a"] = _block_diag([W3[h] for h in range(8)])
    w3b1 = _block_diag([W3[8], W3[9]])
    d["w3b"] = _block_diag([w3b1] * 4)
    # biases, per-partition [128, 1]
    d["b1a"] = b1[:8].reshape(128, 1).copy()
    d["b1b"] = np.tile(b1[8:].reshape(-1), 4).reshape(128, 1)
    d["b2a"] = b2[:8].reshape(128, 1).copy()
    d["b2b"] = np.tile(b2[8:].reshape(-1), 4).reshape(128, 1)
    d["b3a"] = b3[:8].reshape(128, 1).copy()
    d["b3b"] = np.tile(b3[8:].reshape(-1), 4).reshape(128, 1)
    return {
        k: np.ascontiguousarray(v)
        if v.dtype != np.float32
        else np.ascontiguousarray(v, dtype=np.float32)
        for k, v in d.items()
    }


def _kernel_body(tc, outs, ins, repeat=1):
    nc = tc.nc
    # outputs go on the second HWDGE ring (ACT sequencer) so input and
    # output DMA descriptor streams run in parallel
    odma = nc.scalar if os.environ.get("K_OUT_ENG", "scalar") == "scalar" else nc.sync
    strip = os.environ.get("K_STRIP", "")
    bf16_in = os.environ.get("K_IN", "f32pe") == "bf16"
    BF16 = mybir.dt.bfloat16
    outA, outB = outs["outA"], outs["outB"]  # [496,128,128], [31,4,128,128]
    xm = ins.get("xm")  # [PAD, 128] (f32 path)
    xl = ins["xlb"] if os.environ.get("K_IN", "f32pe") == "bf16" else ins.get("xl")
    relu = mybir.ActivationFunctionType.Relu
    copyf = mybir.ActivationFunctionType.Copy

    with ExitStack() as ctx:
        const = ctx.enter_context(tc.tile_pool(name="const", bufs=1))

        ident = const.tile([128, 128], F32)
        make_identity(nc, ident)

        def ld(name, shape, dt=F32):
            t = const.tile(shape, dt, name=name)
            nc.sync.dma_start(t, ins[name])
            return t

        if bf16_in:
            wla = ld("wla_h", [1, 128], BF16)
            wlbs = [None] * 4
            for c in range(4):
                wlbs[c] = const.tile([1, 128], BF16, name=f"wlbsh{c}")
                nc.sync.dma_start(wlbs[c], ins["wlbs_h"][c : c + 1, :])
        else:
            wla = ld("wla", [1, 128], F32R)
            wlbs = [None] * 4
            for c in range(4):
                wlbs[c] = const.tile([1, 128], F32R, name=f"wlbs{c}")
                nc.sync.dma_start(wlbs[c], ins["wlbs"][c : c + 1, :])
        if bf16_in:
            w1a_h = ld("w1a_h", [128, 128], BF16)
            w1a_l = ld("w1a_l", [128, 128], BF16)
            w1bs_h = [None] * 4
            w1bs_l = [None] * 4
            for c in range(4):
                w1bs_h[c] = const.tile([128, 128], BF16, name=f"w1bsh{c}")
                nc.sync.dma_start(w1bs_h[c], ins["w1bs_h"][c])
                w1bs_l[c] = const.tile([128, 128], BF16, name=f"w1bsl{c}")
                nc.sync.dma_start(w1bs_l[c], ins["w1bs_l"][c])
        else:
            w1a = ld("w1a", [128, 128], F32R)
            w1bs = [None] * 4
            for c in range(4):
                w1bs[c] = const.tile([128, 128], F32R, name=f"w1bs{c}")
                nc.sync.dma_start(w1bs[c], ins["w1bs"][c])
        w2a = ld("w2a", [128, 128], F32R)
        w2b = ld("w2b", [128, 128], F32R)
        w3a = ld("w3a", [128, 128], F32R)
        w3b = ld("w3b", [128, 128], F32R)
        b1a = ld("b1a", [128, 1])
        b1b = ld("b1b", [128, 1])
        b2a = ld("b2a", [128, 1])
        b2b = ld("b2b", [128, 1])
        b3a = ld("b3a", [128, 1])
        b3b = ld("b3b", [128, 1])

        io_pool = ctx.enter_context(tc.tile_pool(name="io", bufs=3))
        xt_pool = ctx.enter_context(tc.tile_pool(name="xt", bufs=6))
        h_pool = ctx.enter_context(tc.tile_pool(name="h", bufs=3))
        bsb_pool = ctx.enter_context(tc.tile_pool(name="bsb", bufs=2))
        xl_pool = ctx.enter_context(tc.tile_pool(name="xlp", bufs=2))
        ppool = ctx.enter_context(tc.tile_pool(name="ppool", space="PSUM", bufs=2))
        papool = ctx.enter_context(tc.tile_pool(name="papool", space="PSUM", bufs=2))
        pbpool = ctx.enter_context(tc.tile_pool(name="pbpool", space="PSUM", bufs=2))
        ptpool = ctx.enter_context(tc.tile_pool(name="ptpool", space="PSUM", bufs=2))

        if repeat > 1:
            # timing-only variant: run the whole body `repeat` times on
            # device so single-dispatch wall time isolates device exec
            ctx.enter_context(tc.For_i(0, repeat, 1))

        if strip == "dmaonly":
            for s in range(N_SUPERS):
                for c in range(4):
                    t = s * 4 + c
                    xm_t = io_pool.tile([128, 512], F32, tag="xm")
                    src_ap = xm[t * TILE : (t + 1) * TILE, :].rearrange(
                        "(c p) f -> p c f", p=128
                    )
                    nc.sync.dma_start(xm_t.rearrange("p (c f) -> p c f", c=4), src_ap)
                    odma.dma_start(
                        outA[t * 4 : (t + 1) * 4, :, :].rearrange("j p f -> p j f"),
                        xm_t.rearrange("p (j f) -> p j f", j=4),
                    )
                ob = io_pool.tile([128, TILE], F32, tag="ob")
                nc.vector.tensor_copy(ob, xm_t)
                odma.dma_start(
                    outB[s].rearrange("j p f -> p j f"),
                    ob.rearrange("p (j f) -> p j f", j=4),
                )
            return

        for s in range(N_SUPERS):
            # xlast row for this super-tile: [1, 2048] straight from DRAM
            xlt = xl_pool.tile([1, SUPER], BF16 if bf16_in else F32R, tag="xlt")
            nc.sync.dma_start(xlt, xl[:, s * SUPER : (s + 1) * SUPER])

            pb1 = pbpool.tile([128, TILE], F32, tag="pb")
            xts = []
            for c in range(4):
                t = s * 4 + c
                xls = xlt[:, c * TILE : (c + 1) * TILE]
                pa1 = papool.tile([128, TILE], F32, tag="pa")
                if bf16_in:
                    # hardware DMA-transpose loads of the bf16 hi/lo split:
                    # x = xh + xl to ~2^-17; L1 = xh*Wh + xl*Wh + xh*Wl
                    # (+ the fp32r K=1 last-feature accumulate)
                    xh_t = xt_pool.tile([128, TILE], BF16, tag="xh")
                    nc.sync.dma_start(
                        xh_t, ins["xmh"][t * TILE : (t + 1) * TILE, :], transpose=True
                    )
                    xl_t = xt_pool.tile([128, TILE], BF16, tag="xlo")
                    nc.sync.dma_start(
                        xl_t, ins["xml"][t * TILE : (t + 1) * TILE, :], transpose=True
                    )
                    nc.tensor.matmul(pa1, w1a_h, xh_t, start=True, stop=False)
                    nc.tensor.matmul(pa1, w1a_h, xl_t, start=False, stop=False)
                    nc.tensor.matmul(pa1, w1a_l, xh_t, start=False, stop=False)
                    nc.tensor.matmul(pa1, wla, xls, start=False, stop=True)
                else:
                    # natural load: [128 part, (c4, f128)]
                    xm_t = io_pool.tile([128, 512], F32, tag="xm")
                    src = xm[t * TILE : (t + 1) * TILE, :].rearrange(
                        "(c p) f -> p c f", p=128
                    )
                    nc.sync.dma_start(xm_t.rearrange("p (c f) -> p c f", c=4), src)

                    # transpose 4 chunks of [128,128] -> [feat, batch]
                    if strip == "notrans":
                        xt = xt_pool.tile([128, TILE], F32R, tag="xt")
                        nc.scalar.activation(xt, xm_t, copyf)
                    else:
                        px = ppool.tile([128, TILE], F32, tag="px")
                        for j in range(4):
                            nc.tensor.transpose(
                                px[:, j * 128 : (j + 1) * 128],
                                xm_t[:, j * 128 : (j + 1) * 128],
                                ident,
                            )
                        xt = xt_pool.tile([128, TILE], F32R, tag="xt")
                        nc.scalar.activation(xt, px, copyf)
                    nc.tensor.matmul(pa1, w1a, xt, start=True, stop=False)
                    nc.tensor.matmul(pa1, wla, xls, start=False, stop=True)
                h1a = h_pool.tile([128, TILE], F32R, tag="h1a")
                nc.scalar.activation(h1a, pa1, relu, bias=b1a)

                pa2 = papool.tile([128, TILE], F32, tag="pa")
                nc.tensor.matmul(
                    pa2, w2a, h1a, start=True, stop=True
                )
                h2a = h_pool.tile([128, TILE], F32R, tag="h2a")
                nc.scalar.activation(h2a, pa2, relu, bias=b2a)

                pa3 = papool.tile([128, TILE], F32, tag="pa")
                nc.tensor.matmul(
                    pa3, w3a, h2a, start=True, stop=True
                )
                outta = h_pool.tile([128, TILE], F32, tag="outta")
                nc.vector.tensor_scalar_add(outta, pa3, b3a)

                # transpose back to batch-major and store
                if strip == "notrans":
                    oa = outta
                else:
                    pt = ptpool.tile([128, TILE], F32, tag="pt")
                    for j in range(4):
                        nc.tensor.transpose(
                            pt[:, j * 128 : (j + 1) * 128],
                            outta[:, j * 128 : (j + 1) * 128],
                            ident,
                        )
                    oa = io_pool.tile([128, TILE], F32, tag="oa")
                    nc.vector.tensor_copy(oa, pt)
                odma.dma_start(
                    outA[t * 4 : (t + 1) * 4, :, :].rearrange("j p f -> p j f"),
                    oa.rearrange("p (j f) -> p j f", j=4),
                )

                # ---- B group layer 1: full-M matmul with zero-padded
                # weights accumulating chunk c's strip into packed psum ----
                if bf16_in:
                    nc.tensor.matmul(pb1, w1bs_h[c], xh_t, start=(c == 0), stop=False)
                    nc.tensor.matmul(pb1, w1bs_h[c], xl_t, start=False, stop=False)
                    nc.tensor.matmul(pb1, w1bs_l[c], xh_t, start=False, stop=False)
                    nc.tensor.matmul(pb1, wlbs[c], xls, start=False, stop=(c == 3))
                else:
                    nc.tensor.matmul(pb1, w1bs[c], xt, start=(c == 0), stop=False)
                    nc.tensor.matmul(pb1, wlbs[c], xls, start=False, stop=(c == 3))

            # ---- B group layers 2..3, packed [128=(c,g,i), 512] ----
            h1b = bsb_pool.tile([128, TILE], F32R, tag="h1b")
            nc.scalar.activation(h1b, pb1, relu, bias=b1b)

            pb2 = pbpool.tile([128, TILE], F32, tag="pb")
            nc.tensor.matmul(
                pb2, w2b, h1b, start=True, stop=True
            )
            h2b = bsb_pool.tile([128, TILE], F32R, tag="h2b")
            nc.scalar.activation(h2b, pb2, relu, bias=b2b)

            pb3 = pbpool.tile([128, TILE], F32, tag="pb")
            nc.tensor.matmul(
                pb3, w3b, h2b, start=True, stop=True
            )
            outtb = bsb_pool.tile([128, TILE], F32, tag="outtb")
            nc.vector.tensor_scalar_add(outtb, pb3, b3b)

            if strip == "notrans":
                ob = outtb
            else:
                ptb = ptpool.tile([128, TILE], F32, tag="pt")
                for j in range(4):
                    nc.tensor.transpose(
                        ptb[:, j * 128 : (j + 1) * 128],
                        outtb[:, j * 128 : (j + 1) * 128],
                        ident,
                    )
                ob = io_pool.tile([128, TILE], F32, tag="ob")
                nc.vector.tensor_copy(ob, ptb)
            odma.dma_start(
                outB[s].rearrange("j p f -> p j f"),
                ob.rearrange("p (j f) -> p j f", j=4),
            )


def _make_in_maps(x, W1, b1, W2, b2, W3, b3):
    """Per-core input maps for the currently selected (env) path."""
    import ml_dtypes

    bf16_in = os.environ.get("K_IN", "f32pe") == "bf16"
    wp = _pack_weights(W1, b1, W2, b2, W3, b3)
    x3 = np.asarray(x, np.float32).reshape(N_CORES, SHARD, IN_DIM)
    in_maps = []
    for c in range(N_CORES):
        if bf16_in:
            xmain = x3[c, :, :128]
            xh = np.zeros((PAD, 128), ml_dtypes.bfloat16)
            xh[:SHARD] = xmain.astype(ml_dtypes.bfloat16)
            xlo = np.zeros((PAD, 128), ml_dtypes.bfloat16)
            xlo[:SHARD] = (xmain - xh[:SHARD].astype(np.float32)).astype(
                ml_dtypes.bfloat16
            )
            xlb = np.zeros((1, PAD), ml_dtypes.bfloat16)
            xlb[0, :SHARD] = x3[c, :, 128].astype(ml_dtypes.bfloat16)
            m = {"xmh": xh, "xml": xlo, "xlb": xlb}
        else:
            xm = np.zeros((PAD, 128), np.float32)
            xm[:SHARD] = x3[c, :, :128]
            xl = np.zeros((1, PAD), np.float32)
            xl[0, :SHARD] = x3[c, :, 128]
            m = {"xm": xm, "xl": xl}
        m.update(wp)
        in_maps.append(m)
    return in_maps


_CACHE = {}


def _build(repeat=1):
    key = (
        repeat,
        os.environ.get("K_OUT_ENG", "scalar"),
        os.environ.get("K_STRIP", ""),
        os.environ.get("K_IN", "f32pe"),
    )
    if key in _CACHE:
        return _CACHE[key]
    bf16_in = os.environ.get("K_IN", "f32pe") == "bf16"
    nc = bacc.Bacc(
        "TRN2",
        target_bir_lowering=False,
        debug=False,
        num_devices=N_CORES,
    )
    ins = {}
    if bf16_in:
        BF16 = mybir.dt.bfloat16
        ins["xlb"] = nc.dram_tensor("xlb", (1, PAD), BF16, kind="ExternalInput").ap()
        ins["wla_h"] = nc.dram_tensor("wla_h", (1, 128), BF16, kind="ExternalInput").ap()
        ins["wlbs_h"] = nc.dram_tensor("wlbs_h", (4, 128), BF16, kind="ExternalInput").ap()
        ins["xmh"] = nc.dram_tensor("xmh", (PAD, 128), BF16, kind="ExternalInput").ap()
        ins["xml"] = nc.dram_tensor("xml", (PAD, 128), BF16, kind="ExternalInput").ap()
        ins["w1a_h"] = nc.dram_tensor("w1a_h", (128, 128), BF16, kind="ExternalInput").ap()
        ins["w1a_l"] = nc.dram_tensor("w1a_l", (128, 128), BF16, kind="ExternalInput").ap()
        ins["w1bs_h"] = nc.dram_tensor("w1bs_h", (4, 128, 128), BF16, kind="ExternalInput").ap()
        ins["w1bs_l"] = nc.dram_tensor("w1bs_l", (4, 128, 128), BF16, kind="ExternalInput").ap()
    else:
        ins["xm"] = nc.dram_tensor("xm", (PAD, 128), F32, kind="ExternalInput").ap()
        ins["xl"] = nc.dram_tensor("xl", (1, PAD), F32R, kind="ExternalInput").ap()
    names = [
        ("w2a", (128, 128)),
        ("w2b", (128, 128)),
        ("w3a", (128, 128)),
        ("w3b", (128, 128)),
        ("b1a", (128, 1)),
        ("b1b", (128, 1)),
        ("b2a", (128, 1)),
        ("b2b", (128, 1)),
        ("b3a", (128, 1)),
        ("b3b", (128, 1)),
    ]
    if not bf16_in:
        names += [("w1a", (128, 128)), ("w1bs", (4, 128, 128)),
                  ("wla", (1, 128)), ("wlbs", (4, 128))]
    for name, shape in names:
        dt = F32R if name.startswith("w") else F32
        ins[name] = nc.dram_tensor(name, shape, dt, kind="ExternalInput").ap()
    outs = {
        "outA": nc.dram_tensor(
            "outA", (N_TILES * 4, 128, 128), F32, kind="ExternalOutput"
        ).ap(),
        "outB": nc.dram_tensor(
            "outB", (N_SUPERS, 4, 128, 128), F32, kind="ExternalOutput"
        ).ap(),
    }
    with tile.TileContext(nc) as tc:
        _kernel_body(tc, outs, ins, repeat=repeat)
    nc.compile()
    _CACHE[key] = nc
    return nc


def kernel(x, W1, b1, W2, b2, W3, b3, _want_trace=False):
    x = np.asarray(x, np.float32)
    wp = _pack_weights(W1, b1, W2, b2, W3, b3)

    bf16_in = os.environ.get("K_IN", "f32pe") == "bf16"
    import ml_dtypes

    x3 = x.reshape(N_CORES, SHARD, IN_DIM)
    in_maps = []
    for c in range(N_CORES):
        if bf16_in:
            xlb = np.zeros((1, PAD), ml_dtypes.bfloat16)
            xlb[0, :SHARD] = x3[c, :, 128].astype(ml_dtypes.bfloat16)
            m = {"xlb": xlb}
        else:
            xl = np.zeros((1, PAD), np.float32)
            xl[0, :SHARD] = x3[c, :, 128]
            m = {"xl": xl}
        if bf16_in:
            xmain = x3[c, :, :128]
            xh = np.zeros((PAD, 128), ml_dtypes.bfloat16)
            xh[:SHARD] = xmain.astype(ml_dtypes.bfloat16)
            xlo = np.zeros((PAD, 128), ml_dtypes.bfloat16)
            xlo[:SHARD] = (xmain - xh[:SHARD].astype(np.float32)).astype(
                ml_dtypes.bfloat16
            )
            m["xmh"] = xh
            m["xml"] = xlo
        else:
            xm = np.zeros((PAD, 128), np.float32)
            xm[:SHARD] = x3[c, :, :128]
            m["xm"] = xm
        m.update(wp)
        in_maps.append(m)
    drop = (
        ("w1a_h", "w1a_l", "w1bs_h", "w1bs_l", "wla_h", "wlbs_h")
        if not bf16_in
        else ("w1a", "w1bs", "wla", "wlbs")
    )
    for k in drop:
        for m in in_maps:
            m.pop(k, None)

    nc = _build()
    res = run_bass_kernel_spmd(
        nc, in_maps, core_ids=list(range(N_CORES)), trace=_want_trace
    )

    out = np.empty((NHEADS, BATCH, SKIP), np.float32)
    for c in range(N_CORES):
        oa = res.results[c]["outA"]  # [496, 128, 128]
        ob = res.results[c]["outB"]  # [31, 4, 128, 128]
        # A: [q, b, (h, o)] -> sample = q*128 + b
        a = oa.reshape(PAD, 8, SKIP).transpose(1, 0, 2)
        out[:8, c * SHARD : (c + 1) * SHARD] = a[:, :SHARD]
        # B: [s, j, b, (cc, g, o)] -> sample = s*2048 + cc*512 + j*128 + b
        b = ob.reshape(N_SUPERS, 4, 128, 4, 2, SKIP)
        b = b.transpose(4, 0, 3, 1, 2, 5).reshape(2, PAD, SKIP)
        out[8:, c * SHARD : (c + 1) * SHARD] = b[:, :SHARD]
    if _want_trace:
        kernel.last_results = res
    return out


# revision 27
# speedup vs baseline: 1.6213x; 1.6213x over previous
"""Trainium2 Bass kernel for nn_EnsembleNet (10-head MLP ensemble).

Math (per head h):
  h1 = relu(x @ W1[h] + b1[h])      x: [B, 129], W1: [129, 16]
  h2 = relu(h1 @ W2[h] + b2[h])     W2: [16, 16]
  out[h] = h2 @ W3[h] + b3[h]       W3: [16, 16] -> [10, B, 16]

Strategy (data parallel over 8 cores, B=500000 -> 62500/core, padded to
63488 = 124 tiles x 512 samples):
  - Host splits x into xm=[B,128] (features 0..127) and xl=[1,B] (feature
    128) so the 129-feature contraction becomes one K=128 matmul plus one
    K=1 accumulate whose moving operand loads directly as a [1, 512] row.
  - Heads 0-7 ("A", 8*16=128 outputs) computed in the transposed domain:
    PE-transpose x tiles to [128 feat, 512 batch], then chained block-
    diagonal matmuls (float32r, 1 cyc/row) with fused bias+relu on ACT,
    PE-transpose the result back to batch-major and DMA out.
  - Heads 8-9 ("B", 32 outputs) are packed 4 sub-tiles deep on the
    partition axis (4 x 32 = 128) per 2048-sample super-tile so vector
    ops and matmuls run at full width; layer 1 places each sub-tile's
    strip via zero-padded M=128 weight variants accumulating into one
    packed psum bank (fp32r matmuls require dst base partition 0).
  - Device writes custom layouts outA [496,128,128] / outB [31,4,128,128]
    (>=512B contiguous per DMA descriptor); host permutes (64B-granular)
    into [10, B, 16].
  - Inputs load on the SP HWDGE ring, outputs store on the ACT HWDGE
    ring; one merged DMA per tile each way.
  Measured: ~553 us/core on 8 axon trn2 cores (repeat-loop difference
  method), scale-relative absmax err 2.8e-4 vs fp32 reference.
"""

import os
from contextlib import ExitStack

import numpy as np

import concourse.bass as bass
import concourse.mybir as mybir
import concourse.tile as tile
from concourse import bacc
from concourse.bass_utils import run_bass_kernel_spmd
from concourse.masks import make_identity

F32 = mybir.dt.float32
F32R = mybir.dt.float32r

N_CORES = 8
BATCH = 500000
SHARD = BATCH // N_CORES  # 62500
TILE = 512  # samples per tile (psum bank = 512 f32)
SUPER = 4 * TILE  # 2048, B-group packing unit
N_TILES = 124  # ceil(62500/512) -> pad to 124
PAD = N_TILES * TILE  # 63488
N_SUPERS = N_TILES // 4  # 31

NHEADS = 10
HID = 16
SKIP = 16
IN_DIM = 129


def _block_diag(mats):
    n = len(mats)
    r, c = mats[0].shape
    out = np.zeros((n * r, n * c), dtype=mats[0].dtype)
    for i, m in enumerate(mats):
        out[i * r : (i + 1) * r, i * c : (i + 1) * c] = m
    return out


def _pack_weights(W1, b1, W2, b2, W3, b3):
    """Host-side packing into the SBUF layouts the kernel expects."""
    W1 = np.asarray(W1, np.float32)
    W2 = np.asarray(W2, np.float32)
    W3 = np.asarray(W3, np.float32)
    b1 = np.asarray(b1, np.float32)
    b2 = np.asarray(b2, np.float32)
    b3 = np.asarray(b3, np.float32)

    d = {}
    # L1 A: lhsT [K=128 feat, M=128 (h,o)]
    d["w1a"] = np.ascontiguousarray(W1[:8, :128, :].transpose(1, 0, 2).reshape(128, 128))
    d["wla"] = np.ascontiguousarray(W1[:8, 128, :].reshape(1, 128))
    # L1 B: zero-padded M=128 variants, chunk c owns columns c*32..(c+1)*32
    # (fp32r matmul requires dst psum base partition 0, so each chunk's
    # [K,32] result is placed via its weight columns instead of col-tiling)
    w1b32 = W1[8:, :128, :].transpose(1, 0, 2).reshape(128, 32)
    wlb32 = W1[8:, 128, :].reshape(32)
    w1bs = np.zeros((4, 128, 128), np.float32)
    wlbs = np.zeros((4, 128), np.float32)
    for c in range(4):
        w1bs[c, :, c * 32 : (c + 1) * 32] = w1b32
        wlbs[c, c * 32 : (c + 1) * 32] = wlb32
    d["w1bs"] = w1bs
    d["wlbs"] = wlbs
    # bf16 hi/lo splits for the DMA-transpose input path
    import ml_dtypes

    def split16(m):
        hi = m.astype(ml_dtypes.bfloat16)
        lo = (m - hi.astype(np.float32)).astype(ml_dtypes.bfloat16)
        return hi, lo

    d["w1a_h"], d["w1a_l"] = split16(d["w1a"])
    d["w1bs_h"], d["w1bs_l"] = split16(w1bs)
    d["wla_h"] = d["wla"].astype(ml_dtypes.bfloat16)
    d["wlbs_h"] = wlbs.astype(ml_dtypes.bfloat16)
    # L2: block diag [in (h,i), out (h,o)]
    d["w2a"] = _block_diag([W2[h] for h in range(8)])
    w2b1 = _block_diag([W2[8], W2[9]])  # [32, 32]
    d["w2b"] = _block_diag([w2b1] * 4)  # [128, 128] over (c, g)
    d["w3a"] = _block_diag([W3[h] for h in range(8)])
    w3b1 = _block_diag([W3[8], W3[9]])
    d["w3b"] = _block_diag([w3b1] * 4)
    # biases, per-partition [128, 1]
    d["b1a"] = b1[:8].reshape(128, 1).copy()
    d["b1b"] = np.tile(b1[8:].reshape(-1), 4).reshape(128, 1)
    d["b2a"] = b2[:8].reshape(128, 1).copy()
    d["b2b"] = np.tile(b2[8:].reshape(-1), 4).reshape(128, 1)
    d["b3a"] = b3[:8].reshape(128, 1).copy()
    d["b3b"] = np.tile(b3[8:].reshape(-1), 4).reshape(128, 1)
    return {
        k: np.ascontiguousarray(v)
        if v.dtype != np.float32
        else np.ascontiguousarray(v, dtype=np.float32)
        for k, v in d.items()
    }


def _kernel_body(tc, outs, ins, repeat=1):
    nc = tc.nc
    # outputs go on the second HWDGE ring (ACT sequencer) so input and
    # output DMA descriptor streams run in parallel
    odma = nc.scalar if os.environ.get("K_OUT_ENG", "scalar") == "scalar" else nc.sync
    strip = os.environ.get("K_STRIP", "")
    bf16_in = os.environ.get("K_IN", "f32pe") == "bf16"
    BF16 = mybir.dt.bfloat16
    outA, outB = outs["outA"], outs["outB"]  # [496,128,128], [31,4,128,128]
    xm = ins.get("xm")  # [PAD, 128] (f32 path)
    xl = ins["xlb"] if os.environ.get("K_IN", "f32pe") == "bf16" else ins.get("xl")
    relu = mybir.ActivationFunctionType.Relu
    copyf = mybir.ActivationFunctionType.Copy

    with ExitStack() as ctx:
        const = ctx.enter_context(tc.tile_pool(name="const", bufs=1))

        ident = const.tile([128, 128], F32)
        make_identity(nc, ident)

        def ld(name, shape, dt=F32):
            t = const.tile(shape, dt, name=name)
            nc.sync.dma_start(t, ins[name])
            return t

        if bf16_in:
            wla = ld("wla_h", [1, 128], BF16)
            wlbs = [None] * 4
            for c in range(4):
                wlbs[c] = const.tile([1, 128], BF16, name=f"wlbsh{c}")
                nc.sync.dma_start(wlbs[c], ins["wlbs_h"][c : c + 1, :])
        else:
            wla = ld("wla", [1, 128], F32R)
            wlbs = [None] * 4
            for c in range(4):
                wlbs[c] = const.tile([1, 128], F32R, name=f"wlbs{c}")
                nc.sync.dma_start(wlbs[c], ins["wlbs"][c : c + 1, :])
        if bf16_in:
            w1a_h = ld("w1a_h", [128, 128], BF16)
            w1a_l = ld("w1a_l", [128, 128], BF16)
            w1bs_h = [None] * 4
            w1bs_l = [None] * 4
            for c in range(4):
                w1bs_h[c] = const.tile([128, 128], BF16, name=f"w1bsh{c}")
                nc.sync.dma_start(w1bs_h[c], ins["w1bs_h"][c])
                w1bs_l[c] = const.tile([128, 128], BF16, name=f"w1bsl{c}")
                nc.sync.dma_start(w1bs_l[c], ins["w1bs_l"][c])
        else:
            w1a = ld("w1a", [128, 128], F32R)
            w1bs = [None] * 4
            for c in range(4):
                w1bs[c] = const.tile([128, 128], F32R, name=f"w1bs{c}")
                nc.sync.dma_start(w1bs[c], ins["w1bs"][c])
        w2a = ld("w2a", [128, 128], F32R)
        w2b = ld("w2b", [128, 128], F32R)
        w3a = ld("w3a", [128, 128], F32R)
        w3b = ld("w3b", [128, 128], F32R)
        b1a = ld("b1a", [128, 1])
        b1b = ld("b1b", [128, 1])
        b2a = ld("b2a", [128, 1])
        b2b = ld("b2b", [128, 1])
        b3a = ld("b3a", [128, 1])
        b3b = ld("b3b", [128, 1])

        io_pool = ctx.enter_context(tc.tile_pool(name="io", bufs=3))
        xt_pool = ctx.enter_context(tc.tile_pool(name="xt", bufs=6))
        h_pool = ctx.enter_context(tc.tile_pool(name="h", bufs=3))
        bsb_pool = ctx.enter_context(tc.tile_pool(name="bsb", bufs=2))
        xl_pool = ctx.enter_context(tc.tile_pool(name="xlp", bufs=2))
        ppool = ctx.enter_context(tc.tile_pool(name="ppool", space="PSUM", bufs=2))
        papool = ctx.enter_context(tc.tile_pool(name="papool", space="PSUM", bufs=2))
        pbpool = ctx.enter_context(tc.tile_pool(name="pbpool", space="PSUM", bufs=2))
        ptpool = ctx.enter_context(tc.tile_pool(name="ptpool", space="PSUM", bufs=2))

        if repeat > 1:
            # timing-only variant: run the whole body `repeat` times on
            # device so single-dispatch wall time isolates device exec
            ctx.enter_context(tc.For_i(0, repeat, 1))

        if strip == "dmaonly":
            for s in range(N_SUPERS):
                for c in range(4):
                    t = s * 4 + c
                    xm_t = io_pool.tile([128, 512], F32, tag="xm")
                    src_ap = xm[t * TILE : (t + 1) * TILE, :].rearrange(
                        "(c p) f -> p c f", p=128
                    )
                    nc.sync.dma_start(xm_t.rearrange("p (c f) -> p c f", c=4), src_ap)
                    odma.dma_start(
                        outA[t * 4 : (t + 1) * 4, :, :].rearrange("j p f -> p j f"),
                        xm_t.rearrange("p (j f) -> p j f", j=4),
                    )
                ob = io_pool.tile([128, TILE], F32, tag="ob")
                nc.vector.tensor_copy(ob, xm_t)
                odma.dma_start(
                    outB[s].rearrange("j p f -> p j f"),
                    ob.rearrange("p (j f) -> p j f", j=4),
                )
            return

        if (
            not bf16_in
            and strip == ""
            and os.environ.get("K_PIPE", "pair") == "pair"
        ):
            # stage-major emission over PAIRS of sub-tiles: every engine
            # gets 2 independent ops queued per stage, breaking strict-FIFO
            # head-of-line blocking in the per-tile dependency chain.
            for s in range(N_SUPERS):
                xlt = xl_pool.tile([1, SUPER], F32R, tag="xlt")
                nc.sync.dma_start(xlt, xl[:, s * SUPER : (s + 1) * SUPER])
                pb1 = pbpool.tile([128, TILE], F32, tag="pb")
                for pair in (0, 1):
                    cs = (2 * pair, 2 * pair + 1)
                    st = {c: {} for c in cs}
                    for c in cs:
                        t = s * 4 + c
                        xm_t = io_pool.tile([128, 512], F32, tag="xm", name=f"xm_t{c}")
                        src = xm[t * TILE : (t + 1) * TILE, :].rearrange(
                            "(c p) f -> p c f", p=128
                        )
                        nc.sync.dma_start(
                            xm_t.rearrange("p (c f) -> p c f", c=4), src
                        )
                        st[c]["xm_t"] = xm_t
                    for c in cs:
                        px = ppool.tile([128, TILE], F32, tag="px", name=f"px{c}")
                        for j in range(4):
                            nc.tensor.transpose(
                                px[:, j * 128 : (j + 1) * 128],
                                st[c]["xm_t"][:, j * 128 : (j + 1) * 128],
                                ident,
                            )
                        st[c]["px"] = px
                    for c in cs:
                        xt = xt_pool.tile([128, TILE], F32R, tag="xt", name=f"xt{c}")
                        nc.scalar.activation(xt, st[c]["px"], copyf)
                        st[c]["xt"] = xt
                    for c in cs:
                        xls = xlt[:, c * TILE : (c + 1) * TILE]
                        pa1 = papool.tile([128, TILE], F32, tag="pa", name=f"pa1{c}")
                        nc.tensor.matmul(pa1, w1a, st[c]["xt"], start=True, stop=False)
                        nc.tensor.matmul(pa1, wla, xls, start=False, stop=True)
                        nc.tensor.matmul(
                            pb1, w1bs[c], st[c]["xt"], start=(c == 0), stop=False
                        )
                        nc.tensor.matmul(
                            pb1, wlbs[c], xls, start=False, stop=(c == 3)
                        )
                        st[c]["pa1"] = pa1
                    for c in cs:
                        h1a = h_pool.tile([128, TILE], F32R, tag="h1a", name=f"h1a{c}")
                        nc.scalar.activation(h1a, st[c]["pa1"], relu, bias=b1a)
                        st[c]["h1a"] = h1a
                    for c in cs:
                        pa2 = papool.tile([128, TILE], F32, tag="pa", name=f"pa2{c}")
                        nc.tensor.matmul(pa2, w2a, st[c]["h1a"], start=True, stop=True)
                        st[c]["pa2"] = pa2
                    for c in cs:
                        h2a = h_pool.tile([128, TILE], F32R, tag="h2a", name=f"h2a{c}")
                        nc.scalar.activation(h2a, st[c]["pa2"], relu, bias=b2a)
                        st[c]["h2a"] = h2a
                    for c in cs:
                        pa3 = papool.tile([128, TILE], F32, tag="pa", name=f"pa3{c}")
                        nc.tensor.matmul(pa3, w3a, st[c]["h2a"], start=True, stop=True)
                        st[c]["pa3"] = pa3
                    for c in cs:
                        outta = h_pool.tile(
                            [128, TILE], F32, tag="outta", name=f"outta{c}"
                        )
                        nc.vector.tensor_scalar_add(outta, st[c]["pa3"], b3a)
                        st[c]["outta"] = outta
                    for c in cs:
                        pt = ptpool.tile([128, TILE], F32, tag="pt", name=f"pt{c}")
                        for j in range(4):
                            nc.tensor.transpose(
                                pt[:, j * 128 : (j + 1) * 128],
                                st[c]["outta"][:, j * 128 : (j + 1) * 128],
                                ident,
                            )
                        st[c]["pt"] = pt
                    for c in cs:
                        oa = io_pool.tile([128, TILE], F32, tag="oa", name=f"oa{c}")
                        nc.vector.tensor_copy(oa, st[c]["pt"])
                        st[c]["oa"] = oa
                    for c in cs:
                        t = s * 4 + c
                        odma.dma_start(
                            outA[t * 4 : (t + 1) * 4, :, :].rearrange("j p f -> p j f"),
                            st[c]["oa"].rearrange("p (j f) -> p j f", j=4),
                        )
                # ---- B group layers 2..3 (unchanged) ----
                h1b = bsb_pool.tile([128, TILE], F32R, tag="h1b")
                nc.scalar.activation(h1b, pb1, relu, bias=b1b)
                pb2 = pbpool.tile([128, TILE], F32, tag="pb")
                nc.tensor.matmul(pb2, w2b, h1b, start=True, stop=True)
                h2b = bsb_pool.tile([128, TILE], F32R, tag="h2b")
                nc.scalar.activation(h2b, pb2, relu, bias=b2b)
                pb3 = pbpool.tile([128, TILE], F32, tag="pb")
                nc.tensor.matmul(pb3, w3b, h2b, start=True, stop=True)
                outtb = bsb_pool.tile([128, TILE], F32, tag="outtb")
                nc.vector.tensor_scalar_add(outtb, pb3, b3b)
                ptb = ptpool.tile([128, TILE], F32, tag="pt")
                for j in range(4):
                    nc.tensor.transpose(
                        ptb[:, j * 128 : (j + 1) * 128],
                        outtb[:, j * 128 : (j + 1) * 128],
                        ident,
                    )
                ob = io_pool.tile([128, TILE], F32, tag="ob")
                nc.vector.tensor_copy(ob, ptb)
                odma.dma_start(
                    outB[s].rearrange("j p f -> p j f"),
                    ob.rearrange("p (j f) -> p j f", j=4),
                )
            return

        for s in range(N_SUPERS):
            # xlast row for this super-tile: [1, 2048] straight from DRAM
            xlt = xl_pool.tile([1, SUPER], BF16 if bf16_in else F32R, tag="xlt")
            nc.sync.dma_start(xlt, xl[:, s * SUPER : (s + 1) * SUPER])

            pb1 = pbpool.tile([128, TILE], F32, tag="pb")
            xts = []
            for c in range(4):
                t = s * 4 + c
                xls = xlt[:, c * TILE : (c + 1) * TILE]
                pa1 = papool.tile([128, TILE], F32, tag="pa")
                if bf16_in:
                    # hardware DMA-transpose loads of the bf16 hi/lo split:
                    # x = xh + xl to ~2^-17; L1 = xh*Wh + xl*Wh + xh*Wl
                    # (+ the fp32r K=1 last-feature accumulate)
                    xh_t = xt_pool.tile([128, TILE], BF16, tag="xh")
                    nc.sync.dma_start(
                        xh_t, ins["xmh"][t * TILE : (t + 1) * TILE, :], transpose=True
                    )
                    xl_t = xt_pool.tile([128, TILE], BF16, tag="xlo")
                    nc.sync.dma_start(
                        xl_t, ins["xml"][t * TILE : (t + 1) * TILE, :], transpose=True
                    )
                    nc.tensor.matmul(pa1, w1a_h, xh_t, start=True, stop=False)
                    nc.tensor.matmul(pa1, w1a_h, xl_t, start=False, stop=False)
                    nc.tensor.matmul(pa1, w1a_l, xh_t, start=False, stop=False)
                    nc.tensor.matmul(pa1, wla, xls, start=False, stop=True)
                else:
                    # natural load: [128 part, (c4, f128)]
                    xm_t = io_pool.tile([128, 512], F32, tag="xm")
                    src = xm[t * TILE : (t + 1) * TILE, :].rearrange(
                        "(c p) f -> p c f", p=128
                    )
                    nc.sync.dma_start(xm_t.rearrange("p (c f) -> p c f", c=4), src)

                    # transpose 4 chunks of [128,128] -> [feat, batch]
                    if strip == "notrans":
                        xt = xt_pool.tile([128, TILE], F32R, tag="xt")
                        nc.scalar.activation(xt, xm_t, copyf)
                    else:
                        px = ppool.tile([128, TILE], F32, tag="px")
                        for j in range(4):
                            nc.tensor.transpose(
                                px[:, j * 128 : (j + 1) * 128],
                                xm_t[:, j * 128 : (j + 1) * 128],
                                ident,
                            )
                        xt = xt_pool.tile([128, TILE], F32R, tag="xt")
                        nc.scalar.activation(xt, px, copyf)
                    nc.tensor.matmul(pa1, w1a, xt, start=True, stop=False)
                    nc.tensor.matmul(pa1, wla, xls, start=False, stop=True)
                h1a = h_pool.tile([128, TILE], F32R, tag="h1a")
                nc.scalar.activation(h1a, pa1, relu, bias=b1a)

                pa2 = papool.tile([128, TILE], F32, tag="pa")
                nc.tensor.matmul(
                    pa2, w2a, h1a, start=True, stop=True
                )
                h2a = h_pool.tile([128, TILE], F32R, tag="h2a")
                nc.scalar.activation(h2a, pa2, relu, bias=b2a)

                pa3 = papool.tile([128, TILE], F32, tag="pa")
                nc.tensor.matmul(
                    pa3, w3a, h2a, start=True, stop=True
                )
                outta = h_pool.tile([128, TILE], F32, tag="outta")
                nc.vector.tensor_scalar_add(outta, pa3, b3a)

                # transpose back to batch-major and store
                if strip == "notrans":
                    oa = outta
                else:
                    pt = ptpool.tile([128, TILE], F32, tag="pt")
                    for j in range(4):
                        nc.tensor.transpose(
                            pt[:, j * 128 : (j + 1) * 128],
                            outta[:, j * 128 : (j + 1) * 128],
                            ident,
                        )
                    oa = io_pool.tile([128, TILE], F32, tag="oa")
                    nc.vector.tensor_copy(oa, pt)
                odma.dma_start(
                    outA[t * 4 : (t + 1) * 4, :, :].rearrange("j p f -> p j f"),
                    oa.rearrange("p (j f) -> p j f", j=4),
                )

                # ---- B group layer 1: full-M matmul with zero-padded
                # weights accumulating chunk c's strip into packed psum ----
                if bf16_in:
                    nc.tensor.matmul(pb1, w1bs_h[c], xh_t, start=(c == 0), stop=False)
                    nc.tensor.matmul(pb1, w1bs_h[c], xl_t, start=False, stop=False)
                    nc.tensor.matmul(pb1, w1bs_l[c], xh_t, start=False, stop=False)
                    nc.tensor.matmul(pb1, wlbs[c], xls, start=False, stop=(c == 3))
                else:
                    nc.tensor.matmul(pb1, w1bs[c], xt, start=(c == 0), stop=False)
                    nc.tensor.matmul(pb1, wlbs[c], xls, start=False, stop=(c == 3))

            # ---- B group layers 2..3, packed [128=(c,g,i), 512] ----
            h1b = bsb_pool.tile([128, TILE], F32R, tag="h1b")
            nc.scalar.activation(h1b, pb1, relu, bias=b1b)

            pb2 = pbpool.tile([128, TILE], F32, tag="pb")
            nc.tensor.matmul(
                pb2, w2b, h1b, start=True, stop=True
            )
            h2b = bsb_pool.tile([128, TILE], F32R, tag="h2b")
            nc.scalar.activation(h2b, pb2, relu, bias=b2b)

            pb3 = pbpool.tile([128, TILE], F32, tag="pb")
            nc.tensor.matmul(
                pb3, w3b, h2b, start=True, stop=True
            )
            outtb = bsb_pool.tile([128, TILE], F32, tag="outtb")
            nc.vector.tensor_scalar_add(outtb, pb3, b3b)

            if strip == "notrans":
                ob = outtb
            else:
                ptb = ptpool.tile([128, TILE], F32, tag="pt")
                for j in range(4):
                    nc.tensor.transpose(
                        ptb[:, j * 128 : (j + 1) * 128],
                        outtb[:, j * 128 : (j + 1) * 128],
                        ident,
                    )
                ob = io_pool.tile([128, TILE], F32, tag="ob")
                nc.vector.tensor_copy(ob, ptb)
            odma.dma_start(
                outB[s].rearrange("j p f -> p j f"),
                ob.rearrange("p (j f) -> p j f", j=4),
            )


def _make_in_maps(x, W1, b1, W2, b2, W3, b3):
    """Per-core input maps for the currently selected (env) path."""
    import ml_dtypes

    bf16_in = os.environ.get("K_IN", "f32pe") == "bf16"
    wp = _pack_weights(W1, b1, W2, b2, W3, b3)
    x3 = np.asarray(x, np.float32).reshape(N_CORES, SHARD, IN_DIM)
    in_maps = []
    for c in range(N_CORES):
        if bf16_in:
            xmain = x3[c, :, :128]
            xh = np.zeros((PAD, 128), ml_dtypes.bfloat16)
            xh[:SHARD] = xmain.astype(ml_dtypes.bfloat16)
            xlo = np.zeros((PAD, 128), ml_dtypes.bfloat16)
            xlo[:SHARD] = (xmain - xh[:SHARD].astype(np.float32)).astype(
                ml_dtypes.bfloat16
            )
            xlb = np.zeros((1, PAD), ml_dtypes.bfloat16)
            xlb[0, :SHARD] = x3[c, :, 128].astype(ml_dtypes.bfloat16)
            m = {"xmh": xh, "xml": xlo, "xlb": xlb}
        else:
            xm = np.zeros((PAD, 128), np.float32)
            xm[:SHARD] = x3[c, :, :128]
            xl = np.zeros((1, PAD), np.float32)
            xl[0, :SHARD] = x3[c, :, 128]
            m = {"xm": xm, "xl": xl}
        m.update(wp)
        in_maps.append(m)
    return in_maps


_CACHE = {}


def _build(repeat=1):
    key = (
        repeat,
        os.environ.get("K_OUT_ENG", "scalar"),
        os.environ.get("K_STRIP", ""),
        os.environ.get("K_IN", "f32pe"),
    )
    if key in _CACHE:
        return _CACHE[key]
    bf16_in = os.environ.get("K_IN", "f32pe") == "bf16"
    nc = bacc.Bacc(
        "TRN2",
        target_bir_lowering=False,
        debug=False,
        num_devices=N_CORES,
    )
    ins = {}
    if bf16_in:
        BF16 = mybir.dt.bfloat16
        ins["xlb"] = nc.dram_tensor("xlb", (1, PAD), BF16, kind="ExternalInput").ap()
        ins["wla_h"] = nc.dram_tensor("wla_h", (1, 128), BF16, kind="ExternalInput").ap()
        ins["wlbs_h"] = nc.dram_tensor("wlbs_h", (4, 128), BF16, kind="ExternalInput").ap()
        ins["xmh"] = nc.dram_tensor("xmh", (PAD, 128), BF16, kind="ExternalInput").ap()
        ins["xml"] = nc.dram_tensor("xml", (PAD, 128), BF16, kind="ExternalInput").ap()
        ins["w1a_h"] = nc.dram_tensor("w1a_h", (128, 128), BF16, kind="ExternalInput").ap()
        ins["w1a_l"] = nc.dram_tensor("w1a_l", (128, 128), BF16, kind="ExternalInput").ap()
        ins["w1bs_h"] = nc.dram_tensor("w1bs_h", (4, 128, 128), BF16, kind="ExternalInput").ap()
        ins["w1bs_l"] = nc.dram_tensor("w1bs_l", (4, 128, 128), BF16, kind="ExternalInput").ap()
    else:
        ins["xm"] = nc.dram_tensor("xm", (PAD, 128), F32, kind="ExternalInput").ap()
        ins["xl"] = nc.dram_tensor("xl", (1, PAD), F32R, kind="ExternalInput").ap()
    names = [
        ("w2a", (128, 128)),
        ("w2b", (128, 128)),
        ("w3a", (128, 128)),
        ("w3b", (128, 128)),
        ("b1a", (128, 1)),
        ("b1b", (128, 1)),
        ("b2a", (128, 1)),
        ("b2b", (128, 1)),
        ("b3a", (128, 1)),
        ("b3b", (128, 1)),
    ]
    if not bf16_in:
        names += [("w1a", (128, 128)), ("w1bs", (4, 128, 128)),
                  ("wla", (1, 128)), ("wlbs", (4, 128))]
    for name, shape in names:
        dt = F32R if name.startswith("w") else F32
        ins[name] = nc.dram_tensor(name, shape, dt, kind="ExternalInput").ap()
    outs = {
        "outA": nc.dram_tensor(
            "outA", (N_TILES * 4, 128, 128), F32, kind="ExternalOutput"
        ).ap(),
        "outB": nc.dram_tensor(
            "outB", (N_SUPERS, 4, 128, 128), F32, kind="ExternalOutput"
        ).ap(),
    }
    with tile.TileContext(nc) as tc:
        _kernel_body(tc, outs, ins, repeat=repeat)
    nc.compile()
    _CACHE[key] = nc
    return nc


def kernel(x, W1, b1, W2, b2, W3, b3, _want_trace=False):
    x = np.asarray(x, np.float32)
    wp = _pack_weights(W1, b1, W2, b2, W3, b3)

    bf16_in = os.environ.get("K_IN", "f32pe") == "bf16"
    import ml_dtypes

    x3 = x.reshape(N_CORES, SHARD, IN_DIM)
    in_maps = []
    for c in range(N_CORES):
        if bf16_in:
            xlb = np.zeros((1, PAD), ml_dtypes.bfloat16)
            xlb[0, :SHARD] = x3[c, :, 128].astype(ml_dtypes.bfloat16)
            m = {"xlb": xlb}
        else:
            xl = np.zeros((1, PAD), np.float32)
            xl[0, :SHARD] = x3[c, :, 128]
            m = {"xl": xl}
        if bf16_in:
            xmain = x3[c, :, :128]
            xh = np.zeros((PAD, 128), ml_dtypes.bfloat16)
            xh[:SHARD] = xmain.astype(ml_dtypes.bfloat16)
            xlo = np.zeros((PAD, 128), ml_dtypes.bfloat16)
            xlo[:SHARD] = (xmain - xh[:SHARD].astype(np.float32)).astype(
                ml_dtypes.bfloat16
            )
            m["xmh"] = xh
            m["xml"] = xlo
        else:
            xm = np.zeros((PAD, 128), np.float32)
            xm[:SHARD] = x3[c, :, :128]
            m["xm"] = xm
        m.update(wp)
        in_maps.append(m)
    drop = (
        ("w1a_h", "w1a_l", "w1bs_h", "w1bs_l", "wla_h", "wlbs_h")
        if not bf16_in
        else ("w1a", "w1bs", "wla", "wlbs")
    )
    for k in drop:
        for m in in_maps:
            m.pop(k, None)

    nc = _build()
    res = run_bass_kernel_spmd(
        nc, in_maps, core_ids=list(range(N_CORES)), trace=_want_trace
    )

    out = np.empty((NHEADS, BATCH, SKIP), np.float32)
    for c in range(N_CORES):
        oa = res.results[c]["outA"]  # [496, 128, 128]
        ob = res.results[c]["outB"]  # [31, 4, 128, 128]
        # A: [q, b, (h, o)] -> sample = q*128 + b
        a = oa.reshape(PAD, 8, SKIP).transpose(1, 0, 2)
        out[:8, c * SHARD : (c + 1) * SHARD] = a[:, :SHARD]
        # B: [s, j, b, (cc, g, o)] -> sample = s*2048 + cc*512 + j*128 + b
        b = ob.reshape(N_SUPERS, 4, 128, 4, 2, SKIP)
        b = b.transpose(4, 0, 3, 1, 2, 5).reshape(2, PAD, SKIP)
        out[8:, c * SHARD : (c + 1) * SHARD] = b[:, :SHARD]
    if _want_trace:
        kernel.last_results = res
    return out


# revision 29
# speedup vs baseline: 1.8601x; 1.1473x over previous
"""Trainium2 Bass kernel for nn_EnsembleNet (10-head MLP ensemble).

Math (per head h):
  h1 = relu(x @ W1[h] + b1[h])      x: [B, 129], W1: [129, 16]
  h2 = relu(h1 @ W2[h] + b2[h])     W2: [16, 16]
  out[h] = h2 @ W3[h] + b3[h]       W3: [16, 16] -> [10, B, 16]

Strategy (data parallel over 8 cores, B=500000 -> 62500/core, padded to
63488 = 124 tiles x 512 samples):
  - Host splits x into xm=[B,128] (features 0..127) and xl=[1,B] (feature
    128) so the 129-feature contraction becomes one K=128 matmul plus one
    K=1 accumulate whose moving operand loads directly as a [1, 512] row.
  - Heads 0-7 ("A", 8*16=128 outputs) computed in the transposed domain:
    PE-transpose x tiles to [128 feat, 512 batch], then chained block-
    diagonal matmuls (float32r, 1 cyc/row) with fused bias+relu on ACT,
    PE-transpose the result back to batch-major and DMA out.
  - Heads 8-9 ("B", 32 outputs) are packed 4 sub-tiles deep on the
    partition axis (4 x 32 = 128) per 2048-sample super-tile so vector
    ops and matmuls run at full width; layer 1 places each sub-tile's
    strip via zero-padded M=128 weight variants accumulating into one
    packed psum bank (fp32r matmuls require dst base partition 0).
  - Device writes custom layouts outA [496,128,128] / outB [31,4,128,128]
    (>=512B contiguous per DMA descriptor); host permutes (64B-granular)
    into [10, B, 16].
  - Inputs load on the SP HWDGE ring, outputs store on the ACT HWDGE
    ring; one merged DMA per tile each way.
  Measured: ~521 us/core on 8 axon trn2 cores (repeat-loop difference
  method), scale-relative absmax err 2.822e-4 vs fp32 reference.
  (Measured dead ends kept behind env knobs: K_IN=bf16 DMA-transpose
  input path = 916 us; K_PIPE=pair stage-major emission = 565 us.)
"""

import os
from contextlib import ExitStack

import numpy as np

import concourse.bass as bass
import concourse.mybir as mybir
import concourse.tile as tile
from concourse import bacc
from concourse.bass_utils import run_bass_kernel_spmd
from concourse.masks import make_identity

F32 = mybir.dt.float32
F32R = mybir.dt.float32r

N_CORES = 8
BATCH = 500000
SHARD = BATCH // N_CORES  # 62500
TILE = 512  # samples per tile (psum bank = 512 f32)
SUPER = 4 * TILE  # 2048, B-group packing unit
N_TILES = 124  # ceil(62500/512) -> pad to 124
PAD = N_TILES * TILE  # 63488
N_SUPERS = N_TILES // 4  # 31

NHEADS = 10
HID = 16
SKIP = 16
IN_DIM = 129


def _block_diag(mats):
    n = len(mats)
    r, c = mats[0].shape
    out = np.zeros((n * r, n * c), dtype=mats[0].dtype)
    for i, m in enumerate(mats):
        out[i * r : (i + 1) * r, i * c : (i + 1) * c] = m
    return out


def _pack_weights(W1, b1, W2, b2, W3, b3):
    """Host-side packing into the SBUF layouts the kernel expects."""
    W1 = np.asarray(W1, np.float32)
    W2 = np.asarray(W2, np.float32)
    W3 = np.asarray(W3, np.float32)
    b1 = np.asarray(b1, np.float32)
    b2 = np.asarray(b2, np.float32)
    b3 = np.asarray(b3, np.float32)

    d = {}
    # L1 A: lhsT [K=128 feat, M=128 (h,o)]
    d["w1a"] = np.ascontiguousarray(W1[:8, :128, :].transpose(1, 0, 2).reshape(128, 128))
    d["wla"] = np.ascontiguousarray(W1[:8, 128, :].reshape(1, 128))
    # L1 B: zero-padded M=128 variants, chunk c owns columns c*32..(c+1)*32
    # (fp32r matmul requires dst psum base partition 0, so each chunk's
    # [K,32] result is placed via its weight columns instead of col-tiling)
    w1b32 = W1[8:, :128, :].transpose(1, 0, 2).reshape(128, 32)
    wlb32 = W1[8:, 128, :].reshape(32)
    w1bs = np.zeros((4, 128, 128), np.float32)
    wlbs = np.zeros((4, 128), np.float32)
    for c in range(4):
        w1bs[c, :, c * 32 : (c + 1) * 32] = w1b32
        wlbs[c, c * 32 : (c + 1) * 32] = wlb32
    d["w1bs"] = w1bs
    d["wlbs"] = wlbs
    # bf16 hi/lo splits for the DMA-transpose input path
    import ml_dtypes

    def split16(m):
        hi = m.astype(ml_dtypes.bfloat16)
        lo = (m - hi.astype(np.float32)).astype(ml_dtypes.bfloat16)
        return hi, lo

    d["w1a_h"], d["w1a_l"] = split16(d["w1a"])
    d["w1bs_h"], d["w1bs_l"] = split16(w1bs)
    d["wla_h"] = d["wla"].astype(ml_dtypes.bfloat16)
    d["wlbs_h"] = wlbs.astype(ml_dtypes.bfloat16)
    # L2: block diag [in (h,i), out (h,o)]
    d["w2a"] = _block_diag([W2[h] for h in range(8)])
    w2b1 = _block_diag([W2[8], W2[9]])  # [32, 32]
    d["w2b"] = _block_diag([w2b1] * 4)  # [128, 128] over (c, g)
    d["w3a"] = _block_diag([W3[h] for h in range(8)])
    w3b1 = _block_diag([W3[8], W3[9]])
    d["w3b"] = _block_diag([w3b1] * 4)
    # biases, per-partition [128, 1]
    d["b1a"] = b1[:8].reshape(128, 1).copy()
    d["b1b"] = np.tile(b1[8:].reshape(-1), 4).reshape(128, 1)
    d["b2a"] = b2[:8].reshape(128, 1).copy()
    d["b2b"] = np.tile(b2[8:].reshape(-1), 4).reshape(128, 1)
    d["b3a"] = b3[:8].reshape(128, 1).copy()
    d["b3b"] = np.tile(b3[8:].reshape(-1), 4).reshape(128, 1)
    return {
        k: np.ascontiguousarray(v)
        if v.dtype != np.float32
        else np.ascontiguousarray(v, dtype=np.float32)
        for k, v in d.items()
    }


def _kernel_body(tc, outs, ins, repeat=1):
    nc = tc.nc
    # outputs go on the second HWDGE ring (ACT sequencer) so input and
    # output DMA descriptor streams run in parallel
    odma = nc.scalar if os.environ.get("K_OUT_ENG", "scalar") == "scalar" else nc.sync
    strip = os.environ.get("K_STRIP", "")
    bf16_in = os.environ.get("K_IN", "f32pe") == "bf16"
    BF16 = mybir.dt.bfloat16
    outA, outB = outs["outA"], outs["outB"]  # [496,128,128], [31,4,128,128]
    xm = ins.get("xm")  # [PAD, 128] (f32 path)
    xl = ins["xlb"] if os.environ.get("K_IN", "f32pe") == "bf16" else ins.get("xl")
    relu = mybir.ActivationFunctionType.Relu
    copyf = mybir.ActivationFunctionType.Copy

    with ExitStack() as ctx:
        const = ctx.enter_context(tc.tile_pool(name="const", bufs=1))

        ident = const.tile([128, 128], F32)
        make_identity(nc, ident)

        def ld(name, shape, dt=F32):
            t = const.tile(shape, dt, name=name)
            nc.sync.dma_start(t, ins[name])
            return t

        if bf16_in:
            wla = ld("wla_h", [1, 128], BF16)
            wlbs = [None] * 4
            for c in range(4):
                wlbs[c] = const.tile([1, 128], BF16, name=f"wlbsh{c}")
                nc.sync.dma_start(wlbs[c], ins["wlbs_h"][c : c + 1, :])
        else:
            wla = ld("wla", [1, 128], F32R)
            wlbs = [None] * 4
            for c in range(4):
                wlbs[c] = const.tile([1, 128], F32R, name=f"wlbs{c}")
                nc.sync.dma_start(wlbs[c], ins["wlbs"][c : c + 1, :])
        if bf16_in:
            w1a_h = ld("w1a_h", [128, 128], BF16)
            w1a_l = ld("w1a_l", [128, 128], BF16)
            w1bs_h = [None] * 4
            w1bs_l = [None] * 4
            for c in range(4):
                w1bs_h[c] = const.tile([128, 128], BF16, name=f"w1bsh{c}")
                nc.sync.dma_start(w1bs_h[c], ins["w1bs_h"][c])
                w1bs_l[c] = const.tile([128, 128], BF16, name=f"w1bsl{c}")
                nc.sync.dma_start(w1bs_l[c], ins["w1bs_l"][c])
        else:
            w1a = ld("w1a", [128, 128], F32R)
            w1bs = [None] * 4
            for c in range(4):
                w1bs[c] = const.tile([128, 128], F32R, name=f"w1bs{c}")
                nc.sync.dma_start(w1bs[c], ins["w1bs"][c])
        w2a = ld("w2a", [128, 128], F32R)
        w2b = ld("w2b", [128, 128], F32R)
        w3a = ld("w3a", [128, 128], F32R)
        w3b = ld("w3b", [128, 128], F32R)
        b1a = ld("b1a", [128, 1])
        b1b = ld("b1b", [128, 1])
        b2a = ld("b2a", [128, 1])
        b2b = ld("b2b", [128, 1])
        b3a = ld("b3a", [128, 1])
        b3b = ld("b3b", [128, 1])

        io_pool = ctx.enter_context(tc.tile_pool(name="io", bufs=3))
        xt_pool = ctx.enter_context(tc.tile_pool(name="xt", bufs=6))
        h_pool = ctx.enter_context(tc.tile_pool(name="h", bufs=3))
        bsb_pool = ctx.enter_context(tc.tile_pool(name="bsb", bufs=2))
        xl_pool = ctx.enter_context(tc.tile_pool(name="xlp", bufs=2))
        ppool = ctx.enter_context(tc.tile_pool(name="ppool", space="PSUM", bufs=2))
        papool = ctx.enter_context(tc.tile_pool(name="papool", space="PSUM", bufs=2))
        pbpool = ctx.enter_context(tc.tile_pool(name="pbpool", space="PSUM", bufs=2))
        ptpool = ctx.enter_context(tc.tile_pool(name="ptpool", space="PSUM", bufs=2))

        if repeat > 1:
            # timing-only variant: run the whole body `repeat` times on
            # device so single-dispatch wall time isolates device exec
            ctx.enter_context(tc.For_i(0, repeat, 1))

        if strip == "dmaonly":
            for s in range(N_SUPERS):
                for c in range(4):
                    t = s * 4 + c
                    xm_t = io_pool.tile([128, 512], F32, tag="xm")
                    src_ap = xm[t * TILE : (t + 1) * TILE, :].rearrange(
                        "(c p) f -> p c f", p=128
                    )
                    nc.sync.dma_start(xm_t.rearrange("p (c f) -> p c f", c=4), src_ap)
                    odma.dma_start(
                        outA[t * 4 : (t + 1) * 4, :, :].rearrange("j p f -> p j f"),
                        xm_t.rearrange("p (j f) -> p j f", j=4),
                    )
                ob = io_pool.tile([128, TILE], F32, tag="ob")
                nc.vector.tensor_copy(ob, xm_t)
                odma.dma_start(
                    outB[s].rearrange("j p f -> p j f"),
                    ob.rearrange("p (j f) -> p j f", j=4),
                )
            return

        if (
            not bf16_in
            and strip == ""
            and os.environ.get("K_PIPE", "seq") == "pair"
        ):
            # stage-major emission over PAIRS of sub-tiles: every engine
            # gets 2 independent ops queued per stage, breaking strict-FIFO
            # head-of-line blocking in the per-tile dependency chain.
            for s in range(N_SUPERS):
                xlt = xl_pool.tile([1, SUPER], F32R, tag="xlt")
                nc.sync.dma_start(xlt, xl[:, s * SUPER : (s + 1) * SUPER])
                pb1 = pbpool.tile([128, TILE], F32, tag="pb")
                for pair in (0, 1):
                    cs = (2 * pair, 2 * pair + 1)
                    st = {c: {} for c in cs}
                    for c in cs:
                        t = s * 4 + c
                        xm_t = io_pool.tile([128, 512], F32, tag="xm", name=f"xm_t{c}")
                        src = xm[t * TILE : (t + 1) * TILE, :].rearrange(
                            "(c p) f -> p c f", p=128
                        )
                        nc.sync.dma_start(
                            xm_t.rearrange("p (c f) -> p c f", c=4), src
                        )
                        st[c]["xm_t"] = xm_t
                    for c in cs:
                        px = ppool.tile([128, TILE], F32, tag="px", name=f"px{c}")
                        for j in range(4):
                            nc.tensor.transpose(
                                px[:, j * 128 : (j + 1) * 128],
                                st[c]["xm_t"][:, j * 128 : (j + 1) * 128],
                                ident,
                            )
                        st[c]["px"] = px
                    for c in cs:
                        xt = xt_pool.tile([128, TILE], F32R, tag="xt", name=f"xt{c}")
                        nc.scalar.activation(xt, st[c]["px"], copyf)
                        st[c]["xt"] = xt
                    for c in cs:
                        xls = xlt[:, c * TILE : (c + 1) * TILE]
                        pa1 = papool.tile([128, TILE], F32, tag="pa", name=f"pa1{c}")
                        nc.tensor.matmul(pa1, w1a, st[c]["xt"], start=True, stop=False)
                        nc.tensor.matmul(pa1, wla, xls, start=False, stop=True)
                        nc.tensor.matmul(
                            pb1, w1bs[c], st[c]["xt"], start=(c == 0), stop=False
                        )
                        nc.tensor.matmul(
                            pb1, wlbs[c], xls, start=False, stop=(c == 3)
                        )
                        st[c]["pa1"] = pa1
                    for c in cs:
                        h1a = h_pool.tile([128, TILE], F32R, tag="h1a", name=f"h1a{c}")
                        nc.scalar.activation(h1a, st[c]["pa1"], relu, bias=b1a)
                        st[c]["h1a"] = h1a
                    for c in cs:
                        pa2 = papool.tile([128, TILE], F32, tag="pa", name=f"pa2{c}")
                        nc.tensor.matmul(pa2, w2a, st[c]["h1a"], start=True, stop=True)
                        st[c]["pa2"] = pa2
                    for c in cs:
                        h2a = h_pool.tile([128, TILE], F32R, tag="h2a", name=f"h2a{c}")
                        nc.scalar.activation(h2a, st[c]["pa2"], relu, bias=b2a)
                        st[c]["h2a"] = h2a
                    for c in cs:
                        pa3 = papool.tile([128, TILE], F32, tag="pa", name=f"pa3{c}")
                        nc.tensor.matmul(pa3, w3a, st[c]["h2a"], start=True, stop=True)
                        st[c]["pa3"] = pa3
                    for c in cs:
                        outta = h_pool.tile(
                            [128, TILE], F32, tag="outta", name=f"outta{c}"
                        )
                        nc.vector.tensor_scalar_add(outta, st[c]["pa3"], b3a)
                        st[c]["outta"] = outta
                    for c in cs:
                        pt = ptpool.tile([128, TILE], F32, tag="pt", name=f"pt{c}")
                        for j in range(4):
                            nc.tensor.transpose(
                                pt[:, j * 128 : (j + 1) * 128],
                                st[c]["outta"][:, j * 128 : (j + 1) * 128],
                                ident,
                            )
                        st[c]["pt"] = pt
                    for c in cs:
                        oa = io_pool.tile([128, TILE], F32, tag="oa", name=f"oa{c}")
                        nc.vector.tensor_copy(oa, st[c]["pt"])
                        st[c]["oa"] = oa
                    for c in cs:
                        t = s * 4 + c
                        odma.dma_start(
                            outA[t * 4 : (t + 1) * 4, :, :].rearrange("j p f -> p j f"),
                            st[c]["oa"].rearrange("p (j f) -> p j f", j=4),
                        )
                # ---- B group layers 2..3 (unchanged) ----
                h1b = bsb_pool.tile([128, TILE], F32R, tag="h1b")
                nc.scalar.activation(h1b, pb1, relu, bias=b1b)
                pb2 = pbpool.tile([128, TILE], F32, tag="pb")
                nc.tensor.matmul(pb2, w2b, h1b, start=True, stop=True)
                h2b = bsb_pool.tile([128, TILE], F32R, tag="h2b")
                nc.scalar.activation(h2b, pb2, relu, bias=b2b)
                pb3 = pbpool.tile([128, TILE], F32, tag="pb")
                nc.tensor.matmul(pb3, w3b, h2b, start=True, stop=True)
                outtb = bsb_pool.tile([128, TILE], F32, tag="outtb")
                nc.vector.tensor_scalar_add(outtb, pb3, b3b)
                ptb = ptpool.tile([128, TILE], F32, tag="pt")
                for j in range(4):
                    nc.tensor.transpose(
                        ptb[:, j * 128 : (j + 1) * 128],
                        outtb[:, j * 128 : (j + 1) * 128],
                        ident,
                    )
                ob = io_pool.tile([128, TILE], F32, tag="ob")
                nc.vector.tensor_copy(ob, ptb)
                odma.dma_start(
                    outB[s].rearrange("j p f -> p j f"),
                    ob.rearrange("p (j f) -> p j f", j=4),
                )
            return

        for s in range(N_SUPERS):
            # xlast row for this super-tile: [1, 2048] straight from DRAM
            xlt = xl_pool.tile([1, SUPER], BF16 if bf16_in else F32R, tag="xlt")
            nc.sync.dma_start(xlt, xl[:, s * SUPER : (s + 1) * SUPER])

            pb1 = pbpool.tile([128, TILE], F32, tag="pb")
            xts = []
            for c in range(4):
                t = s * 4 + c
                xls = xlt[:, c * TILE : (c + 1) * TILE]
                pa1 = papool.tile([128, TILE], F32, tag="pa")
                if bf16_in:
                    # hardware DMA-transpose loads of the bf16 hi/lo split:
                    # x = xh + xl to ~2^-17; L1 = xh*Wh + xl*Wh + xh*Wl
                    # (+ the fp32r K=1 last-feature accumulate)
                    xh_t = xt_pool.tile([128, TILE], BF16, tag="xh")
                    nc.sync.dma_start(
                        xh_t, ins["xmh"][t * TILE : (t + 1) * TILE, :], transpose=True
                    )
                    xl_t = xt_pool.tile([128, TILE], BF16, tag="xlo")
                    nc.sync.dma_start(
                        xl_t, ins["xml"][t * TILE : (t + 1) * TILE, :], transpose=True
                    )
                    nc.tensor.matmul(pa1, w1a_h, xh_t, start=True, stop=False)
                    nc.tensor.matmul(pa1, w1a_h, xl_t, start=False, stop=False)
                    nc.tensor.matmul(pa1, w1a_l, xh_t, start=False, stop=False)
                    nc.tensor.matmul(pa1, wla, xls, start=False, stop=True)
                else:
                    # natural load: [128 part, (c4, f128)]
                    xm_t = io_pool.tile([128, 512], F32, tag="xm")
                    src = xm[t * TILE : (t + 1) * TILE, :].rearrange(
                        "(c p) f -> p c f", p=128
                    )
                    nc.sync.dma_start(xm_t.rearrange("p (c f) -> p c f", c=4), src)

                    # transpose 4 chunks of [128,128] -> [feat, batch]
                    if strip == "notrans":
                        xt = xt_pool.tile([128, TILE], F32R, tag="xt")
                        nc.scalar.activation(xt, xm_t, copyf)
                    else:
                        px = ppool.tile([128, TILE], F32, tag="px")
                        for j in range(4):
                            nc.tensor.transpose(
                                px[:, j * 128 : (j + 1) * 128],
                                xm_t[:, j * 128 : (j + 1) * 128],
                                ident,
                            )
                        xt = xt_pool.tile([128, TILE], F32R, tag="xt")
                        nc.scalar.activation(xt, px, copyf)
                    nc.tensor.matmul(pa1, w1a, xt, start=True, stop=False)
                    nc.tensor.matmul(pa1, wla, xls, start=False, stop=True)
                h1a = h_pool.tile([128, TILE], F32R, tag="h1a")
                nc.scalar.activation(h1a, pa1, relu, bias=b1a)

                pa2 = papool.tile([128, TILE], F32, tag="pa")
                nc.tensor.matmul(
                    pa2, w2a, h1a, start=True, stop=True
                )
                h2a = h_pool.tile([128, TILE], F32R, tag="h2a")
                nc.scalar.activation(h2a, pa2, relu, bias=b2a)

                pa3 = papool.tile([128, TILE], F32, tag="pa")
                nc.tensor.matmul(
                    pa3, w3a, h2a, start=True, stop=True
                )
                outta = h_pool.tile([128, TILE], F32, tag="outta")
                nc.vector.tensor_scalar_add(outta, pa3, b3a)

                # transpose back to batch-major and store
                if strip == "notrans":
                    oa = outta
                else:
                    pt = ptpool.tile([128, TILE], F32, tag="pt")
                    for j in range(4):
                        nc.tensor.transpose(
                            pt[:, j * 128 : (j + 1) * 128],
                            outta[:, j * 128 : (j + 1) * 128],
                            ident,
                        )
                    oa = io_pool.tile([128, TILE], F32, tag="oa")
                    nc.vector.tensor_copy(oa, pt)
                odma.dma_start(
                    outA[t * 4 : (t + 1) * 4, :, :].rearrange("j p f -> p j f"),
                    oa.rearrange("p (j f) -> p j f", j=4),
                )

                # ---- B group layer 1: full-M matmul with zero-padded
                # weights accumulating chunk c's strip into packed psum ----
                if bf16_in:
                    nc.tensor.matmul(pb1, w1bs_h[c], xh_t, start=(c == 0), stop=False)
                    nc.tensor.matmul(pb1, w1bs_h[c], xl_t, start=False, stop=False)
                    nc.tensor.matmul(pb1, w1bs_l[c], xh_t, start=False, stop=False)
                    nc.tensor.matmul(pb1, wlbs[c], xls, start=False, stop=(c == 3))
                else:
                    nc.tensor.matmul(pb1, w1bs[c], xt, start=(c == 0), stop=False)
                    nc.tensor.matmul(pb1, wlbs[c], xls, start=False, stop=(c == 3))

            # ---- B group layers 2..3, packed [128=(c,g,i), 512] ----
            h1b = bsb_pool.tile([128, TILE], F32R, tag="h1b")
            nc.scalar.activation(h1b, pb1, relu, bias=b1b)

            pb2 = pbpool.tile([128, TILE], F32, tag="pb")
            nc.tensor.matmul(
                pb2, w2b, h1b, start=True, stop=True
            )
            h2b = bsb_pool.tile([128, TILE], F32R, tag="h2b")
            nc.scalar.activation(h2b, pb2, relu, bias=b2b)

            pb3 = pbpool.tile([128, TILE], F32, tag="pb")
            nc.tensor.matmul(
                pb3, w3b, h2b, start=True, stop=True
            )
            outtb = bsb_pool.tile([128, TILE], F32, tag="outtb")
            nc.vector.tensor_scalar_add(outtb, pb3, b3b)

            if strip == "notrans":
                ob = outtb
            else:
                ptb = ptpool.tile([128, TILE], F32, tag="pt")
                for j in range(4):
                    nc.tensor.transpose(
                        ptb[:, j * 128 : (j + 1) * 128],
                        outtb[:, j * 128 : (j + 1) * 128],
                        ident,
                    )
                ob = io_pool.tile([128, TILE], F32, tag="ob")
                nc.vector.tensor_copy(ob, ptb)
            odma.dma_start(
                outB[s].rearrange("j p f -> p j f"),
                ob.rearrange("p (j f) -> p j f", j=4),
            )


def _make_in_maps(x, W1, b1, W2, b2, W3, b3):
    """Per-core input maps for the currently selected (env) path."""
    import ml_dtypes

    bf16_in = os.environ.get("K_IN", "f32pe") == "bf16"
    wp = _pack_weights(W1, b1, W2, b2, W3, b3)
    x3 = np.asarray(x, np.float32).reshape(N_CORES, SHARD, IN_DIM)
    in_maps = []
    for c in range(N_CORES):
        if bf16_in:
            xmain = x3[c, :, :128]
            xh = np.zeros((PAD, 128), ml_dtypes.bfloat16)
            xh[:SHARD] = xmain.astype(ml_dtypes.bfloat16)
            xlo = np.zeros((PAD, 128), ml_dtypes.bfloat16)
            xlo[:SHARD] = (xmain - xh[:SHARD].astype(np.float32)).astype(
                ml_dtypes.bfloat16
            )
            xlb = np.zeros((1, PAD), ml_dtypes.bfloat16)
            xlb[0, :SHARD] = x3[c, :, 128].astype(ml_dtypes.bfloat16)
            m = {"xmh": xh, "xml": xlo, "xlb": xlb}
        else:
            xm = np.zeros((PAD, 128), np.float32)
            xm[:SHARD] = x3[c, :, :128]
            xl = np.zeros((1, PAD), np.float32)
            xl[0, :SHARD] = x3[c, :, 128]
            m = {"xm": xm, "xl": xl}
        m.update(wp)
        in_maps.append(m)
    return in_maps


_CACHE = {}


def _build(repeat=1):
    key = (
        repeat,
        os.environ.get("K_OUT_ENG", "scalar"),
        os.environ.get("K_STRIP", ""),
        os.environ.get("K_IN", "f32pe"),
    )
    if key in _CACHE:
        return _CACHE[key]
    bf16_in = os.environ.get("K_IN", "f32pe") == "bf16"
    nc = bacc.Bacc(
        "TRN2",
        target_bir_lowering=False,
        debug=False,
        num_devices=N_CORES,
    )
    ins = {}
    if bf16_in:
        BF16 = mybir.dt.bfloat16
        ins["xlb"] = nc.dram_tensor("xlb", (1, PAD), BF16, kind="ExternalInput").ap()
        ins["wla_h"] = nc.dram_tensor("wla_h", (1, 128), BF16, kind="ExternalInput").ap()
        ins["wlbs_h"] = nc.dram_tensor("wlbs_h", (4, 128), BF16, kind="ExternalInput").ap()
        ins["xmh"] = nc.dram_tensor("xmh", (PAD, 128), BF16, kind="ExternalInput").ap()
        ins["xml"] = nc.dram_tensor("xml", (PAD, 128), BF16, kind="ExternalInput").ap()
        ins["w1a_h"] = nc.dram_tensor("w1a_h", (128, 128), BF16, kind="ExternalInput").ap()
        ins["w1a_l"] = nc.dram_tensor("w1a_l", (128, 128), BF16, kind="ExternalInput").ap()
        ins["w1bs_h"] = nc.dram_tensor("w1bs_h", (4, 128, 128), BF16, kind="ExternalInput").ap()
        ins["w1bs_l"] = nc.dram_tensor("w1bs_l", (4, 128, 128), BF16, kind="ExternalInput").ap()
    else:
        ins["xm"] = nc.dram_tensor("xm", (PAD, 128), F32, kind="ExternalInput").ap()
        ins["xl"] = nc.dram_tensor("xl", (1, PAD), F32R, kind="ExternalInput").ap()
    names = [
        ("w2a", (128, 128)),
        ("w2b", (128, 128)),
        ("w3a", (128, 128)),
        ("w3b", (128, 128)),
        ("b1a", (128, 1)),
        ("b1b", (128, 1)),
        ("b2a", (128, 1)),
        ("b2b", (128, 1)),
        ("b3a", (128, 1)),
        ("b3b", (128, 1)),
    ]
    if not bf16_in:
        names += [("w1a", (128, 128)), ("w1bs", (4, 128, 128)),
                  ("wla", (1, 128)), ("wlbs", (4, 128))]
    for name, shape in names:
        dt = F32R if name.startswith("w") else F32
        ins[name] = nc.dram_tensor(name, shape, dt, kind="ExternalInput").ap()
    outs = {
        "outA": nc.dram_tensor(
            "outA", (N_TILES * 4, 128, 128), F32, kind="ExternalOutput"
        ).ap(),
        "outB": nc.dram_tensor(
            "outB", (N_SUPERS, 4, 128, 128), F32, kind="ExternalOutput"
        ).ap(),
    }
    with tile.TileContext(nc) as tc:
        _kernel_body(tc, outs, ins, repeat=repeat)
    nc.compile()
    _CACHE[key] = nc
    return nc


def kernel(x, W1, b1, W2, b2, W3, b3, _want_trace=False):
    x = np.asarray(x, np.float32)
    wp = _pack_weights(W1, b1, W2, b2, W3, b3)

    bf16_in = os.environ.get("K_IN", "f32pe") == "bf16"
    import ml_dtypes

    x3 = x.reshape(N_CORES, SHARD, IN_DIM)
    in_maps = []
    for c in range(N_CORES):
        if bf16_in:
            xlb = np.zeros((1, PAD), ml_dtypes.bfloat16)
            xlb[0, :SHARD] = x3[c, :, 128].astype(ml_dtypes.bfloat16)
            m = {"xlb": xlb}
        else:
            xl = np.zeros((1, PAD), np.float32)
            xl[0, :SHARD] = x3[c, :, 128]
            m = {"xl": xl}
        if bf16_in:
            xmain = x3[c, :, :128]
            xh = np.zeros((PAD, 128), ml_dtypes.bfloat16)
            xh[:SHARD] = xmain.astype(ml_dtypes.bfloat16)
            xlo = np.zeros((PAD, 128), ml_dtypes.bfloat16)
            xlo[:SHARD] = (xmain - xh[:SHARD].astype(np.float32)).astype(
                ml_dtypes.bfloat16
            )
            m["xmh"] = xh
            m["xml"] = xlo
        else:
            xm = np.zeros((PAD, 128), np.float32)
            xm[:SHARD] = x3[c, :, :128]
            m["xm"] = xm
        m.update(wp)
        in_maps.append(m)
    drop = (
        ("w1a_h", "w1a_l", "w1bs_h", "w1bs_l", "wla_h", "wlbs_h")
        if not bf16_in
        else ("w1a", "w1bs", "wla", "wlbs")
    )
    for k in drop:
        for m in in_maps:
            m.pop(k, None)

    nc = _build()
    res = run_bass_kernel_spmd(
        nc, in_maps, core_ids=list(range(N_CORES)), trace=_want_trace
    )

    out = np.empty((NHEADS, BATCH, SKIP), np.float32)
    for c in range(N_CORES):
        oa = res.results[c]["outA"]  # [496, 128, 128]
        ob = res.results[c]["outB"]  # [31, 4, 128, 128]
        # A: [q, b, (h, o)] -> sample = q*128 + b
        a = oa.reshape(PAD, 8, SKIP).transpose(1, 0, 2)
        out[:8, c * SHARD : (c + 1) * SHARD] = a[:, :SHARD]
        # B: [s, j, b, (cc, g, o)] -> sample = s*2048 + cc*512 + j*128 + b
        b = ob.reshape(N_SUPERS, 4, 128, 4, 2, SKIP)
        b = b.transpose(4, 0, 3, 1, 2, 5).reshape(2, PAD, SKIP)
        out[8:, c * SHARD : (c + 1) * SHARD] = b[:, :SHARD]
    if _want_trace:
        kernel.last_results = res
    return out
